# revision 5
# baseline (speedup 1.0000x reference)
"""Single-head attention (B=4, T=4096, D_IN=1024, D_HEAD=D_OUT=64) on 8 TRN2
NeuronCores.

Sharding: core c handles batch b = c//2 and query-half h = c%2 (2048 queries),
computing K/V for the full sequence of its batch redundantly on both cores of
a pair.  Inputs are pre-transposed/permuted on the host so the device program
is identical on every core (SPMD):

  xt[c]  = x[b].T  with columns permuted so the core's own query-half comes
           first.  The s-permutation of K/V is harmless (softmax + weighted
           sum are permutation-invariant); queries come out in natural order.

Device pipeline per core (all matmuls via TensorE, fp32r rounding mode):
  1. qT = Wq.T @ x.T (own half),  [kT; vT] = [Wk|Wv].T @ x.T (full seq)
  2. v_aug[s,0:64] = v (PE-transpose of vT), v_aug[s,64] = 1.0
  3. per query-chunk: scoresT[s,tq] = kT.T-free matmul;  ACT exp(scale*x)
     PSUM->SBUF;  out_augT[o,tq] += v_aug.T @ expT  (row 64 = softmax denom)
  4. PE-transpose out_augT, multiply by reciprocal of denom, DMA out.
"""

import numpy as np

import concourse.bacc as bacc
import concourse.bass as bass
import concourse.mybir as mybir
import concourse.tile as tile
from concourse.bass_utils import run_bass_kernel_spmd

B, T, D_IN, D_HEAD, D_OUT = 4, 4096, 1024, 64, 64
N_CORES = 8
TQ = T // 2          # queries per core
ND = D_IN // 128     # contraction chunks (8)
NS = T // 128        # key/value chunks of 128 (32)
NT2 = TQ // 512      # 512-wide query chunks per core (4)
SCALE = 1.0 / np.sqrt(np.float32(D_HEAD))

F32 = mybir.dt.float32
F32R = mybir.dt.float32r


def build_program(dt_mm=F32R):
    """Build the SPMD Bass program. dt_mm: matmul operand dtype (F32R or F32)."""
    nc = bacc.Bacc("TRN2", target_bir_lowering=False, debug=False,
                   num_devices=N_CORES)

    xt_d = nc.dram_tensor("xt", [D_IN, T], dt_mm, kind="ExternalInput").ap()
    wkv_d = nc.dram_tensor("wkv", [D_IN, 128], dt_mm, kind="ExternalInput").ap()
    wq_d = nc.dram_tensor("wq", [D_IN, 64], dt_mm, kind="ExternalInput").ap()
    bkv_d = nc.dram_tensor("bkv", [128, 1], F32, kind="ExternalInput").ap()
    bq_d = nc.dram_tensor("bq", [64, 1], F32, kind="ExternalInput").ap()
    id_d = nc.dram_tensor("ident", [128, 128], dt_mm, kind="ExternalInput").ap()
    out_d = nc.dram_tensor("out", [TQ, 64], F32, kind="ExternalOutput").ap()

    EXP = mybir.ActivationFunctionType.Exp

    with tile.TileContext(nc) as tc:
        with (
            tc.tile_pool(name="const", bufs=1) as cpool,
            tc.tile_pool(name="xt", bufs=3) as xpool,
            tc.tile_pool(name="proj", bufs=1) as ppool,
            tc.tile_pool(name="exp", bufs=3) as epool,
            tc.tile_pool(name="outp", bufs=1) as opool,
        ):
            # ---- constants ----
            wkv_sb = cpool.tile([128, ND, 128], dt_mm)
            wq_sb = cpool.tile([128, ND, 64], dt_mm)
            bkv_sb = cpool.tile([128, 1], F32)
            bq_sb = cpool.tile([64, 1], F32)
            id_sb = cpool.tile([128, 128], dt_mm)
            nc.sync.dma_start(wkv_sb[:], wkv_d.rearrange("(c p) h -> p c h", p=128))
            nc.sync.dma_start(wq_sb[:], wq_d.rearrange("(c p) h -> p c h", p=128))
            nc.sync.dma_start(bkv_sb[:], bkv_d[:])
            nc.sync.dma_start(bq_sb[:], bq_d[:])
            nc.sync.dma_start(id_sb[:], id_d[:])

            # ---- projection outputs ----
            kvT_sb = ppool.tile([128, T], dt_mm)     # rows 0:64 kT, 64:128 vT
            qT_sb = ppool.tile([64, TQ], dt_mm)
            vaug_sb = ppool.tile([128, NS, 65], dt_mm)

            # ---- phase A: projections ----
            with tc.tile_pool(name="psA", bufs=1, space="PSUM") as psA:
                for half in range(2):
                    pkv = [psA.tile([128, 512], F32, tag=f"pkv{t2}", name=f"pkv{half}_{t2}")
                           for t2 in range(4)]
                    pq = ([psA.tile([64, 512], F32, tag=f"pq{t2}", name=f"pq{t2}")
                           for t2 in range(4)] if half == 0 else None)
                    for d in range(ND):
                        xt_t = xpool.tile([128, 2048], dt_mm, tag="xt", name=f"xt{half}_{d}")
                        nc.sync.dma_start(
                            xt_t[:], xt_d[d * 128:(d + 1) * 128,
                                          half * 2048:(half + 1) * 2048])
                        for t2 in range(4):
                            rhs = xt_t[:, t2 * 512:(t2 + 1) * 512]
                            nc.tensor.matmul(pkv[t2][:], wkv_sb[:, d, :], rhs,
                                             start=(d == 0), stop=(d == ND - 1))
                            if half == 0:
                                nc.tensor.matmul(pq[t2][:], wq_sb[:, d, :], rhs,
                                                 start=(d == 0), stop=(d == ND - 1))
                    for t2 in range(4):
                        s0 = half * 2048 + t2 * 512
                        nc.vector.tensor_scalar_add(
                            kvT_sb[:, s0:s0 + 512], pkv[t2][:], bkv_sb[:])
                        if half == 0:
                            nc.vector.tensor_scalar_add(
                                qT_sb[:, t2 * 512:(t2 + 1) * 512], pq[t2][:], bq_sb[:])

            # ---- phase B: v_aug = [v | ones] via PE transpose of vT ----
            nc.vector.memset(vaug_sb[:].bitcast(F32), 1.0)
            with tc.tile_pool(name="psB", bufs=2, space="PSUM") as psB:
                for c in range(NS):
                    pvt = psB.tile([128, 64], dt_mm, tag="pvt", name=f"pvt{c}")
                    nc.tensor.transpose(pvt[:], kvT_sb[64:128, c * 128:(c + 1) * 128],
                                        id_sb[64:128, 64:128])
                    nc.vector.tensor_copy(vaug_sb[:, c, 0:64], pvt[:])

            # ---- phases C+D: attention per 1024-wide query chunk ----
            osb = opool.tile([128, TQ // 128, 64], F32)
            with (
                tc.tile_pool(name="psC", bufs=1, space="PSUM") as psC,
                tc.tile_pool(name="psD", bufs=2, space="PSUM") as psD,
            ):
                for tqc in range(TQ // 1024):
                    po = psC.tile([65, 1024], F32, tag="po", name=f"po{tqc}")
                    for s in range(NS):
                        ps_t = psC.tile([128, 1024], F32, tag="ps", bufs=2,
                                        name=f"ps{tqc}_{s}")
                        for j in range(2):
                            nc.tensor.matmul(
                                ps_t[:, j * 512:(j + 1) * 512],
                                kvT_sb[0:64, s * 128:(s + 1) * 128],
                                qT_sb[:, tqc * 1024 + j * 512:tqc * 1024 + (j + 1) * 512],
                                start=True, stop=True)
                        et = epool.tile([128, 1024], dt_mm, tag="et", name=f"et{tqc}_{s}")
                        nc.scalar.activation(et[:], ps_t[:], EXP, scale=float(SCALE))
                        for j in range(2):
                            nc.tensor.matmul(
                                po[:, j * 512:(j + 1) * 512],
                                vaug_sb[:, s, :],
                                et[:, j * 512:(j + 1) * 512],
                                start=(s == 0), stop=(s == NS - 1))
                    # finalize this query chunk
                    oT = opool.tile([65, 1024], F32, tag="oT", bufs=2, name=f"oT{tqc}")
                    nc.vector.tensor_copy(oT[:], po[:])
                    for j in range(8):
                        jj = tqc * 8 + j
                        pt = psD.tile([128, 65], F32, tag="pt", name=f"pt{jj}")
                        nc.tensor.transpose(pt[:], oT[:, j * 128:(j + 1) * 128],
                                            id_sb[0:65, 0:65].bitcast(F32))
                        rec = opool.tile([128, 1], F32, tag="rec", bufs=2, name=f"rec{jj}")
                        nc.vector.reciprocal(rec[:], pt[:, 64:65])
                        nc.vector.tensor_scalar_mul(osb[:, jj, :], pt[:, 0:64], rec[:])
            nc.sync.dma_start(out_d.rearrange("(j p) o -> p j o", p=128), osb[:])

    nc.compile()
    return nc


_PROGRAM_CACHE = {}


def get_program(dt_mm=F32R):
    key = str(dt_mm)
    if key not in _PROGRAM_CACHE:
        _PROGRAM_CACHE[key] = build_program(dt_mm)
    return _PROGRAM_CACHE[key]


def make_in_maps(x, Wk, bk, Wq, bq, Wv, bv):
    x = np.asarray(x, dtype=np.float32)
    wkv = np.ascontiguousarray(np.concatenate([Wk, Wv], axis=1), dtype=np.float32)
    wq = np.ascontiguousarray(Wq, dtype=np.float32)
    bkv = np.concatenate([bk, bv]).astype(np.float32).reshape(128, 1)
    bqv = np.asarray(bq, dtype=np.float32).reshape(64, 1)
    ident = np.eye(128, dtype=np.float32)
    in_maps = []
    for c in range(N_CORES):
        b, half = c // 2, c % 2
        xb = x[b]
        own = xb[half * TQ:(half + 1) * TQ].T
        other = xb[(1 - half) * TQ:(2 - half) * TQ].T
        xt = np.ascontiguousarray(np.concatenate([own, other], axis=1))
        in_maps.append({"xt": xt, "wkv": wkv, "wq": wq, "bkv": bkv,
                        "bq": bqv, "ident": ident})
    return in_maps


def assemble(results):
    out = np.empty((B, T, D_OUT), dtype=np.float32)
    for c in range(N_CORES):
        b, half = c // 2, c % 2
        out[b, half * TQ:(half + 1) * TQ, :] = results[c]["out"]
    return out


def kernel(x, Wk, bk, Wq, bq, Wv, bv):
    nc = get_program()
    in_maps = make_in_maps(x, Wk, bk, Wq, bq, Wv, bv)
    res = run_bass_kernel_spmd(nc, in_maps, list(range(N_CORES)))
    return assemble(res.results)


# revision 17
# speedup vs baseline: 236.2313x; 236.2313x over previous
"""Single-head attention (B=4, T=4096, D_IN=1024, D_HEAD=D_OUT=64) on 8 TRN2
NeuronCores.

Sharding: core c handles batch b = c//2 and query-half h = c%2 (2048 queries),
computing K/V for the full sequence of its batch redundantly on both cores of
a pair.  Inputs are pre-transposed/permuted on the host so the device program
is identical on every core (SPMD):

  xt[c]  = x[b].T  with columns permuted so the core's own query-half comes
           first.  The s-permutation of K/V is harmless (softmax + weighted
           sum are permutation-invariant); queries come out in natural order.

Device pipeline per core (all matmuls on TensorE in fp32r rounding mode):
  A. qT = Wq.T @ x.T (own half),  [kT; vT] = [Wk|Wv].T @ x.T (full seq)
  B. v_aug[s,0:64] = v (PE-transpose of vT), v_aug[s,64] = 1.0
  C. per query-chunk: scoresT[s,tq] = kT(s-chunk) x qT;  ACT exp(scale*x)
     PSUM->SBUF;  out_augT[o,tq] += v_aug.T @ expT  (row 64 = softmax denom)
  D. PE-transpose out_augT, scale rows by reciprocal of denom, DMA out.
"""

import numpy as np

import concourse.bacc as bacc
import concourse.bass as bass
import concourse.mybir as mybir
import concourse.tile as tile
from concourse.bass_utils import run_bass_kernel_spmd

B, T, D_IN, D_HEAD, D_OUT = 4, 4096, 1024, 64, 64
N_CORES = 8
TQ = T // 2          # queries per core
ND = D_IN // 128     # contraction chunks (8)
NS = T // 128        # key/value chunks of 128 (32)
SCALE = float(1.0 / np.sqrt(np.float32(D_HEAD)))

F32 = mybir.dt.float32
F32R = mybir.dt.float32r
EXPF = mybir.ActivationFunctionType.Exp


def emit_body(nc, tc, io, dt_mm, phases="ABCD", n_iters=None):
    """Emit the per-core kernel body. io: dict of DRAM APs.

    Single scheduling window: projections for the second sequence half (and
    their DMAs) overlap the ACT-bound attention over the first half.  PSUM
    budget (8 banks): pa(2x1) + ps(2x2) + po(1x2) = 8; phase-B transposes and
    phase-D transposes borrow the 'pa'/'ps' slots respectively.
    """
    xt_d, wkv_d, wq_d = io["xt"], io["wkv"], io["wq"]
    bkv_d, bq_d, id_d, out_d = io["bkv"], io["bq"], io["ident"], io["out"]

    with (
        tc.tile_pool(name="const", bufs=1) as cpool,
        tc.tile_pool(name="xt", bufs=6) as xpool,
        tc.tile_pool(name="proj", bufs=1) as ppool,
        tc.tile_pool(name="exp", bufs=2) as epool,
        tc.tile_pool(name="outp", bufs=1) as opool,
        tc.tile_pool(name="psum", bufs=1, space="PSUM") as qpool,
    ):
        # ---- constants ----
        wkv_sb = cpool.tile([128, ND, 128], dt_mm)
        wq_sb = cpool.tile([128, ND, 64], dt_mm)
        bkv_sb = cpool.tile([128, 1], F32)
        bq_sb = cpool.tile([64, 1], F32)
        id_sb = cpool.tile([128, 128], dt_mm)
        nc.scalar.dma_start(wkv_sb[:], wkv_d.rearrange("(c p) h -> p c h", p=128))
        nc.scalar.dma_start(wq_sb[:], wq_d.rearrange("(c p) h -> p c h", p=128))
        nc.gpsimd.dma_start(bkv_sb[:], bkv_d[:])
        nc.gpsimd.dma_start(bq_sb[:], bq_d[:])
        nc.gpsimd.dma_start(id_sb[:], id_d[:])

        kvT_sb = ppool.tile([128, T], dt_mm)     # rows 0:64 kT, 64:128 vT
        qT_sb = ppool.tile([64, TQ], dt_mm)
        vaug_sb = ppool.tile([128, NS, 65], dt_mm)
        osb = opool.tile([128, TQ // 128, 64], F32)
        if "D" not in phases:
            nc.vector.memset(osb[:], 0.0)

        def body():
            nc.vector.memset(vaug_sb[:].bitcast(F32), 1.0)

            # ---- phase A+B: projections, one PSUM bank per accumulation ----
            # One 2MB DMA per (half, t2) pass, alternating HWDGE rings.
            xt_tiles = {}

            def load_tile(half, t2):
                xt_t = xpool.tile([128, ND, 512], dt_mm, tag="xt",
                                  name=f"xt{half}_{t2}")
                src = xt_d[:, half * 2048 + t2 * 512:
                           half * 2048 + (t2 + 1) * 512]
                eng = nc.sync if (half * 4 + t2) % 2 == 0 else nc.scalar
                eng.dma_start(xt_t[:], src.rearrange("(c p) t -> p c t", p=128))
                xt_tiles[(half, t2)] = xt_t

            def q_pass(t2):
                pq = qpool.tile([64, 512], F32, tag="pa", bufs=2, name=f"pq{t2}")
                for d in range(ND):
                    nc.tensor.matmul(pq[:], wq_sb[:, d, :],
                                     xt_tiles[(0, t2)][:, d, :],
                                     start=(d == 0), stop=(d == ND - 1))
                nc.vector.tensor_scalar_add(
                    qT_sb[:, t2 * 512:(t2 + 1) * 512], pq[:], bq_sb[:])

            def kv_pass(half, t2):
                pkv = qpool.tile([128, 512], F32, tag="pa", bufs=2,
                                 name=f"pkv{half}_{t2}")
                for d in range(ND):
                    nc.tensor.matmul(pkv[:], wkv_sb[:, d, :],
                                     xt_tiles[(half, t2)][:, d, :],
                                     start=(d == 0), stop=(d == ND - 1))
                s0 = half * 2048 + t2 * 512
                nc.vector.tensor_scalar_add(kvT_sb[:, s0:s0 + 512], pkv[:], bkv_sb[:])
                if "B" in phases:
                    for c in range(s0 // 128, s0 // 128 + 4):
                        pvt = qpool.tile([128, 64], dt_mm, tag="pa", bufs=2,
                                         name=f"pvt{c}")
                        nc.tensor.transpose(
                            pvt[:], kvT_sb[64:128, c * 128:(c + 1) * 128],
                            id_sb[64:128, 64:128])
                        nc.vector.tensor_copy(vaug_sb[:, c, 0:64], pvt[:])

            pos = {}

            def attn_group(tqc, g):
                # 4 consecutive s-chunks of the attention pipeline for query
                # chunk tqc, emitted as soon as their K/V chunks exist.
                if "C" not in phases:
                    return
                if tqc not in pos:
                    pos[tqc] = qpool.tile([65, 1024], F32, tag="po", bufs=1,
                                          name=f"po{tqc}")
                po = pos[tqc]
                for s in range(4 * g, 4 * g + 4):
                    ps_t = qpool.tile([128, 1024], F32, tag="ps", bufs=2,
                                      name=f"ps{tqc}_{s}")
                    for j in range(2):
                        nc.tensor.matmul(
                            ps_t[:, j * 512:(j + 1) * 512],
                            kvT_sb[0:64, s * 128:(s + 1) * 128],
                            qT_sb[:, tqc * 1024 + j * 512:
                                  tqc * 1024 + (j + 1) * 512],
                            start=True, stop=True)
                    et = epool.tile([128, 1024], dt_mm, tag="et",
                                    name=f"et{tqc}_{s}")
                    nc.scalar.activation(et[:], ps_t[:], EXPF, scale=SCALE)
                    for j in range(2):
                        nc.tensor.matmul(
                            po[:, j * 512:(j + 1) * 512],
                            vaug_sb[:, s, :],
                            et[:, j * 512:(j + 1) * 512],
                            start=(s == 0), stop=(s == NS - 1))

            def finish(tqc):
                if "C" not in phases or "D" not in phases:
                    return
                oT = opool.tile([65, 1024], F32, tag="oT", bufs=2, name=f"oT{tqc}")
                nc.vector.tensor_copy(oT[:], pos[tqc][:])
                for j in range(8):
                    jj = tqc * 8 + j
                    pt = qpool.tile([128, 65], F32, tag="pa", bufs=2, name=f"pt{jj}")
                    nc.tensor.transpose(pt[:], oT[:, j * 128:(j + 1) * 128],
                                        id_sb[0:65, 0:65].bitcast(F32))
                    rec = opool.tile([128, 1], F32, tag="rec", bufs=2,
                                     name=f"rec{jj}")
                    nc.vector.reciprocal(rec[:], pt[:, 64:65])
                    nc.vector.tensor_scalar_mul(osb[:, jj, :], pt[:, 0:64], rec[:])

            for t2 in range(4):
                load_tile(0, t2)
            for t2 in range(4):
                load_tile(1, t2)
            # interleave attention (tqc=0) with K/V production so ACT starts
            # as soon as the first K/V chunks and qT[0:1024] exist
            q_pass(0)
            kv_pass(0, 0)
            q_pass(1)
            kv_pass(0, 1)
            attn_group(0, 0)
            q_pass(2)
            kv_pass(0, 2)
            attn_group(0, 1)
            q_pass(3)
            kv_pass(0, 3)
            attn_group(0, 2)
            for t2 in range(4):
                kv_pass(1, t2)
                attn_group(0, 3 + t2)
            attn_group(0, 7)
            for g in range(8):
                attn_group(1, g)
            finish(0)
            finish(1)

        if n_iters is None:
            body()
        else:
            with tc.For_i(0, n_iters, 1) as _i:
                body()
        nc.sync.dma_start(out_d.rearrange("(j p) o -> p j o", p=128), osb[:])


def build_program(dt_mm=F32R, phases="ABCD", n_iters=None):
    nc = bacc.Bacc("TRN2", target_bir_lowering=False, debug=False,
                   num_devices=N_CORES)
    io = {
        "xt": nc.dram_tensor("xt", [D_IN, T], dt_mm, kind="ExternalInput").ap(),
        "wkv": nc.dram_tensor("wkv", [D_IN, 128], dt_mm, kind="ExternalInput").ap(),
        "wq": nc.dram_tensor("wq", [D_IN, 64], dt_mm, kind="ExternalInput").ap(),
        "bkv": nc.dram_tensor("bkv", [128, 1], F32, kind="ExternalInput").ap(),
        "bq": nc.dram_tensor("bq", [64, 1], F32, kind="ExternalInput").ap(),
        "ident": nc.dram_tensor("ident", [128, 128], dt_mm, kind="ExternalInput").ap(),
        "out": nc.dram_tensor("out", [TQ, 64], F32, kind="ExternalOutput").ap(),
    }
    with tile.TileContext(nc) as tc:
        emit_body(nc, tc, io, dt_mm, phases=phases, n_iters=n_iters)
    nc.compile()
    return nc


_PROGRAM_CACHE = {}


def get_program(dt_mm=F32R):
    key = str(dt_mm)
    if key not in _PROGRAM_CACHE:
        _PROGRAM_CACHE[key] = build_program(dt_mm)
    return _PROGRAM_CACHE[key]


def make_in_maps(x, Wk, bk, Wq, bq, Wv, bv):
    x = np.asarray(x, dtype=np.float32)
    wkv = np.ascontiguousarray(np.concatenate([Wk, Wv], axis=1), dtype=np.float32)
    wq = np.ascontiguousarray(Wq, dtype=np.float32)
    bkv = np.concatenate([bk, bv]).astype(np.float32).reshape(128, 1)
    bqv = np.asarray(bq, dtype=np.float32).reshape(64, 1)
    ident = np.eye(128, dtype=np.float32)
    in_maps = []
    for c in range(N_CORES):
        b, half = c // 2, c % 2
        xb = x[b]
        own = xb[half * TQ:(half + 1) * TQ].T
        other = xb[(1 - half) * TQ:(2 - half) * TQ].T
        xt = np.ascontiguousarray(np.concatenate([own, other], axis=1))
        in_maps.append({"xt": xt, "wkv": wkv, "wq": wq, "bkv": bkv,
                        "bq": bqv, "ident": ident})
    return in_maps


def assemble(results):
    out = np.empty((B, T, D_OUT), dtype=np.float32)
    for c in range(N_CORES):
        b, half = c // 2, c % 2
        out[b, half * TQ:(half + 1) * TQ, :] = results[c]["out"]
    return out


def kernel(x, Wk, bk, Wq, bq, Wv, bv):
    nc = get_program()
    in_maps = make_in_maps(x, Wk, bk, Wq, bq, Wv, bv)
    res = run_bass_kernel_spmd(nc, in_maps, list(range(N_CORES)))
    return assemble(res.results)


# revision 21
# speedup vs baseline: 244.5312x; 1.0351x over previous
"""Single-head attention (B=4, T=4096, D_IN=1024, D_HEAD=D_OUT=64) on 8 TRN2
NeuronCores.

Sharding: core c handles batch b = c//2 and query-half h = c%2 (2048 queries),
computing K/V for the full sequence of its batch redundantly on both cores of
a pair.  Inputs are pre-transposed/permuted on the host so the device program
is identical on every core (SPMD):

  xt[c]  = x[b].T  with columns permuted so the core's own query-half comes
           first.  The s-permutation of K/V is harmless (softmax + weighted
           sum are permutation-invariant); queries come out in natural order.

Device pipeline per core (all matmuls on TensorE in fp32r rounding mode):
  A. qT = Wq.T @ x.T (own half),  [kT; vT] = [Wk|Wv].T @ x.T (full seq)
  B. v_aug[s,0:64] = v (PE-transpose of vT), v_aug[s,64] = 1.0
  C. per query-chunk: scoresT[s,tq] = kT(s-chunk) x qT;  ACT exp(scale*x)
     PSUM->SBUF;  out_augT[o,tq] += v_aug.T @ expT  (row 64 = softmax denom)
  D. PE-transpose out_augT, scale rows by reciprocal of denom, DMA out.
"""

import numpy as np

import concourse.bacc as bacc
import concourse.bass as bass
import concourse.mybir as mybir
import concourse.tile as tile
from concourse.bass_utils import run_bass_kernel_spmd

B, T, D_IN, D_HEAD, D_OUT = 4, 4096, 1024, 64, 64
N_CORES = 8
TQ = T // 2          # queries per core
ND = D_IN // 128     # contraction chunks (8)
NS = T // 128        # key/value chunks of 128 (32)
SCALE = float(1.0 / np.sqrt(np.float32(D_HEAD)))

F32 = mybir.dt.float32
F32R = mybir.dt.float32r
EXPF = mybir.ActivationFunctionType.Exp


def emit_body(nc, tc, io, dt_mm, phases="ABCD", n_iters=None):
    """Emit the per-core kernel body. io: dict of DRAM APs.

    Single scheduling window: projections for the second sequence half (and
    their DMAs) overlap the ACT-bound attention over the first half.  PSUM
    budget (8 banks): pa(2x1) + ps(2x2) + po(1x2) = 8; phase-B transposes and
    phase-D transposes borrow the 'pa'/'ps' slots respectively.
    """
    xt_d, wkv_d, wq_d = io["xt"], io["wkv"], io["wq"]
    bkv_d, bq_d, id_d, out_d = io["bkv"], io["bq"], io["ident"], io["out"]

    with (
        tc.tile_pool(name="const", bufs=1) as cpool,
        tc.tile_pool(name="xt", bufs=6) as xpool,
        tc.tile_pool(name="proj", bufs=1) as ppool,
        tc.tile_pool(name="exp", bufs=2) as epool,
        tc.tile_pool(name="outp", bufs=1) as opool,
        tc.tile_pool(name="psum", bufs=1, space="PSUM") as qpool,
    ):
        # ---- constants ----
        wkv_sb = cpool.tile([128, ND, 128], dt_mm)
        wq_sb = cpool.tile([128, ND, 64], dt_mm)
        bkv_sb = cpool.tile([128, 1], F32)
        bq_sb = cpool.tile([64, 1], F32)
        id_sb = cpool.tile([128, 128], dt_mm)
        nc.scalar.dma_start(wkv_sb[:], wkv_d.rearrange("(c p) h -> p c h", p=128))
        nc.scalar.dma_start(wq_sb[:], wq_d.rearrange("(c p) h -> p c h", p=128))
        nc.gpsimd.dma_start(bkv_sb[:], bkv_d[:])
        nc.gpsimd.dma_start(bq_sb[:], bq_d[:])
        nc.gpsimd.dma_start(id_sb[:], id_d[:])

        # per-pass tiles so consumers depend on exactly one producer each
        kvs = [ppool.tile([128, 512], dt_mm, name=f"kvs{i}") for i in range(8)]
        qts = [ppool.tile([64, 512], dt_mm, name=f"qts{i}") for i in range(4)]
        vau = [ppool.tile([128, 65], dt_mm, name=f"vau{i}") for i in range(NS)]
        osb = opool.tile([128, TQ // 128, 64], F32)
        if "D" not in phases:
            nc.vector.memset(osb[:], 0.0)

        def body():
            for s in range(NS):
                nc.gpsimd.memset(vau[s][:, 64:65].bitcast(F32), 1.0)

            # ---- phase A+B: projections, one PSUM bank per accumulation ----
            # One 2MB DMA per (half, t2) pass, alternating HWDGE rings.
            xt_tiles = {}

            def load_tile(half, t2):
                xt_t = xpool.tile([128, ND, 512], dt_mm, tag="xt",
                                  name=f"xt{half}_{t2}")
                src = xt_d[:, half * 2048 + t2 * 512:
                           half * 2048 + (t2 + 1) * 512]
                srcr = src.rearrange("(c p) t -> p c t", p=128)
                eng = nc.sync if (half * 4 + t2) % 2 == 0 else nc.scalar
                eng.dma_start(xt_t[:, 0:ND // 2, :], srcr[:, 0:ND // 2, :])
                eng.dma_start(xt_t[:, ND // 2:ND, :], srcr[:, ND // 2:ND, :])
                xt_tiles[(half, t2)] = xt_t

            def q_pass(t2):
                pq = qpool.tile([64, 512], F32, tag="pa", bufs=2, name=f"pq{t2}")
                for d in range(ND):
                    nc.tensor.matmul(pq[:], wq_sb[:, d, :],
                                     xt_tiles[(0, t2)][:, d, :],
                                     start=(d == 0), stop=(d == ND - 1))
                nc.vector.tensor_scalar_add(qts[t2][:], pq[:], bq_sb[:])

            def kv_pass(half, t2):
                pkv = qpool.tile([128, 512], F32, tag="pa", bufs=2,
                                 name=f"pkv{half}_{t2}")
                for d in range(ND):
                    nc.tensor.matmul(pkv[:], wkv_sb[:, d, :],
                                     xt_tiles[(half, t2)][:, d, :],
                                     start=(d == 0), stop=(d == ND - 1))
                i = half * 4 + t2
                nc.vector.tensor_scalar_add(kvs[i][:], pkv[:], bkv_sb[:])
                if "B" in phases:
                    for cc in range(4):
                        c = i * 4 + cc
                        pvt = qpool.tile([128, 64], dt_mm, tag="pa", bufs=2,
                                         name=f"pvt{c}")
                        nc.tensor.transpose(
                            pvt[:], kvs[i][64:128, cc * 128:(cc + 1) * 128],
                            id_sb[64:128, 64:128])
                        nc.vector.tensor_copy(vau[c][:, 0:64], pvt[:])

            pos = {}

            def attn_group(tqc, g):
                # 4 consecutive s-chunks of the attention pipeline for query
                # chunk tqc, emitted as soon as their K/V chunks exist.
                if "C" not in phases:
                    return
                if tqc not in pos:
                    pos[tqc] = qpool.tile([65, 1024], F32, tag="po", bufs=1,
                                          name=f"po{tqc}")
                po = pos[tqc]
                for s in range(4 * g, 4 * g + 4):
                    ps_t = qpool.tile([128, 1024], F32, tag="ps", bufs=2,
                                      name=f"ps{tqc}_{s}")
                    for j in range(2):
                        nc.tensor.matmul(
                            ps_t[:, j * 512:(j + 1) * 512],
                            kvs[s // 4][0:64, (s % 4) * 128:(s % 4 + 1) * 128],
                            qts[2 * tqc + j][:],
                            start=True, stop=True)
                    et = epool.tile([128, 1024], dt_mm, tag="et",
                                    name=f"et{tqc}_{s}")
                    nc.scalar.activation(et[:], ps_t[:], EXPF, scale=SCALE)
                    for j in range(2):
                        nc.tensor.matmul(
                            po[:, j * 512:(j + 1) * 512],
                            vau[s][:],
                            et[:, j * 512:(j + 1) * 512],
                            start=(s == 0), stop=(s == NS - 1))

            def finish(tqc):
                if "C" not in phases or "D" not in phases:
                    return
                oT = opool.tile([65, 1024], F32, tag="oT", bufs=2, name=f"oT{tqc}")
                nc.vector.tensor_copy(oT[:], pos[tqc][:])
                for j in range(8):
                    jj = tqc * 8 + j
                    pt = qpool.tile([128, 65], F32, tag="pa", bufs=2, name=f"pt{jj}")
                    nc.tensor.transpose(pt[:], oT[:, j * 128:(j + 1) * 128],
                                        id_sb[0:65, 0:65].bitcast(F32))
                    rec = opool.tile([128, 1], F32, tag="rec", bufs=2,
                                     name=f"rec{jj}")
                    nc.vector.reciprocal(rec[:], pt[:, 64:65])
                    nc.vector.tensor_scalar_mul(osb[:, jj, :], pt[:, 0:64], rec[:])

            for t2 in range(4):
                load_tile(0, t2)
            for t2 in range(4):
                load_tile(1, t2)
            # interleave attention (tqc=0) with K/V production so ACT starts
            # as soon as the first K/V chunks and qT[0:1024] exist
            q_pass(0)
            kv_pass(0, 0)
            q_pass(1)
            kv_pass(0, 1)
            attn_group(0, 0)
            q_pass(2)
            kv_pass(0, 2)
            attn_group(0, 1)
            q_pass(3)
            kv_pass(0, 3)
            attn_group(0, 2)
            for t2 in range(4):
                kv_pass(1, t2)
                attn_group(0, 3 + t2)
            attn_group(0, 7)
            for g in range(8):
                attn_group(1, g)
            finish(0)
            finish(1)

        if n_iters is None:
            body()
        else:
            with tc.For_i(0, n_iters, 1) as _i:
                body()
        nc.sync.dma_start(out_d.rearrange("(j p) o -> p j o", p=128), osb[:])


def build_program(dt_mm=F32R, phases="ABCD", n_iters=None):
    nc = bacc.Bacc("TRN2", target_bir_lowering=False, debug=False,
                   num_devices=N_CORES)
    io = {
        "xt": nc.dram_tensor("xt", [D_IN, T], dt_mm, kind="ExternalInput").ap(),
        "wkv": nc.dram_tensor("wkv", [D_IN, 128], dt_mm, kind="ExternalInput").ap(),
        "wq": nc.dram_tensor("wq", [D_IN, 64], dt_mm, kind="ExternalInput").ap(),
        "bkv": nc.dram_tensor("bkv", [128, 1], F32, kind="ExternalInput").ap(),
        "bq": nc.dram_tensor("bq", [64, 1], F32, kind="ExternalInput").ap(),
        "ident": nc.dram_tensor("ident", [128, 128], dt_mm, kind="ExternalInput").ap(),
        "out": nc.dram_tensor("out", [TQ, 64], F32, kind="ExternalOutput").ap(),
    }
    with tile.TileContext(nc) as tc:
        emit_body(nc, tc, io, dt_mm, phases=phases, n_iters=n_iters)
    nc.compile()
    return nc


_PROGRAM_CACHE = {}


def get_program(dt_mm=F32R):
    key = str(dt_mm)
    if key not in _PROGRAM_CACHE:
        _PROGRAM_CACHE[key] = build_program(dt_mm)
    return _PROGRAM_CACHE[key]


def make_in_maps(x, Wk, bk, Wq, bq, Wv, bv):
    x = np.asarray(x, dtype=np.float32)
    wkv = np.ascontiguousarray(np.concatenate([Wk, Wv], axis=1), dtype=np.float32)
    wq = np.ascontiguousarray(Wq, dtype=np.float32)
    bkv = np.concatenate([bk, bv]).astype(np.float32).reshape(128, 1)
    bqv = np.asarray(bq, dtype=np.float32).reshape(64, 1)
    ident = np.eye(128, dtype=np.float32)
    in_maps = []
    for c in range(N_CORES):
        b, half = c // 2, c % 2
        xb = x[b]
        own = xb[half * TQ:(half + 1) * TQ].T
        other = xb[(1 - half) * TQ:(2 - half) * TQ].T
        xt = np.ascontiguousarray(np.concatenate([own, other], axis=1))
        in_maps.append({"xt": xt, "wkv": wkv, "wq": wq, "bkv": bkv,
                        "bq": bqv, "ident": ident})
    return in_maps


def assemble(results):
    out = np.empty((B, T, D_OUT), dtype=np.float32)
    for c in range(N_CORES):
        b, half = c // 2, c % 2
        out[b, half * TQ:(half + 1) * TQ, :] = results[c]["out"]
    return out


def kernel(x, Wk, bk, Wq, bq, Wv, bv):
    nc = get_program()
    in_maps = make_in_maps(x, Wk, bk, Wq, bq, Wv, bv)
    res = run_bass_kernel_spmd(nc, in_maps, list(range(N_CORES)))
    return assemble(res.results)


# revision 22
# speedup vs baseline: 245.8180x; 1.0053x over previous
"""Single-head attention (B=4, T=4096, D_IN=1024, D_HEAD=D_OUT=64) on 8 TRN2
NeuronCores.

Sharding: core c handles batch b = c//2 and query-half h = c%2 (2048 queries),
computing K/V for the full sequence of its batch redundantly on both cores of
a pair.  Inputs are pre-transposed/permuted on the host so the device program
is identical on every core (SPMD):

  xt[c]  = x[b].T  with columns permuted so the core's own query-half comes
           first.  The s-permutation of K/V is harmless (softmax + weighted
           sum are permutation-invariant); queries come out in natural order.

Device pipeline per core (all matmuls on TensorE in fp32r rounding mode):
  A. qT = Wq.T @ x.T (own half),  [kT; vT] = [Wk|Wv].T @ x.T (full seq)
  B. v_aug[s,0:64] = v (PE-transpose of vT), v_aug[s,64] = 1.0
  C. per query-chunk: scoresT[s,tq] = kT(s-chunk) x qT;  ACT exp(scale*x)
     PSUM->SBUF;  out_augT[o,tq] += v_aug.T @ expT  (row 64 = softmax denom)
  D. PE-transpose out_augT, scale rows by reciprocal of denom, DMA out.
"""

import numpy as np

import concourse.bacc as bacc
import concourse.bass as bass
import concourse.mybir as mybir
import concourse.tile as tile
from concourse.bass_utils import run_bass_kernel_spmd

B, T, D_IN, D_HEAD, D_OUT = 4, 4096, 1024, 64, 64
N_CORES = 8
TQ = T // 2          # queries per core
ND = D_IN // 128     # contraction chunks (8)
NS = T // 128        # key/value chunks of 128 (32)
SCALE = float(1.0 / np.sqrt(np.float32(D_HEAD)))

F32 = mybir.dt.float32
F32R = mybir.dt.float32r
EXPF = mybir.ActivationFunctionType.Exp


def emit_body(nc, tc, io, dt_mm, phases="ABCD", n_iters=None):
    """Emit the per-core kernel body. io: dict of DRAM APs.

    Single scheduling window: projections for the second sequence half (and
    their DMAs) overlap the ACT-bound attention over the first half.  PSUM
    budget (8 banks): pa(2x1) + ps(2x2) + po(1x2) = 8; phase-B transposes and
    phase-D transposes borrow the 'pa'/'ps' slots respectively.
    """
    xt_d, wkv_d, wq_d = io["xt"], io["wkv"], io["wq"]
    bkv_d, bq_d, id_d, out_d = io["bkv"], io["bq"], io["ident"], io["out"]

    with (
        tc.tile_pool(name="const", bufs=1) as cpool,
        tc.tile_pool(name="xt", bufs=6) as xpool,
        tc.tile_pool(name="proj", bufs=1) as ppool,
        tc.tile_pool(name="exp", bufs=2) as epool,
        tc.tile_pool(name="outp", bufs=1) as opool,
        tc.tile_pool(name="psum", bufs=1, space="PSUM") as qpool,
    ):
        # ---- constants ----
        wkv_sb = cpool.tile([128, ND, 128], dt_mm)
        wq_sb = cpool.tile([128, ND, 64], dt_mm)
        bkv_sb = cpool.tile([128, 1], F32)
        bq_sb = cpool.tile([64, 1], F32)
        id_sb = cpool.tile([128, 128], dt_mm)
        nc.scalar.dma_start(wq_sb[:], wq_d.rearrange("(c p) h -> p c h", p=128))
        nc.scalar.dma_start(wkv_sb[:], wkv_d.rearrange("(c p) h -> p c h", p=128))
        nc.gpsimd.dma_start(bkv_sb[:], bkv_d[:])
        nc.gpsimd.dma_start(bq_sb[:], bq_d[:])
        nc.gpsimd.dma_start(id_sb[:], id_d[:])

        # per-pass tiles so consumers depend on exactly one producer each
        kvs = [ppool.tile([128, 512], dt_mm, name=f"kvs{i}") for i in range(8)]
        qts = [ppool.tile([64, 512], dt_mm, name=f"qts{i}") for i in range(4)]
        vau = [ppool.tile([128, 65], dt_mm, name=f"vau{i}") for i in range(NS)]
        osb = opool.tile([128, TQ // 128, 64], F32)
        if "D" not in phases:
            nc.vector.memset(osb[:], 0.0)

        def body():
            for s in range(NS):
                nc.gpsimd.memset(vau[s][:, 64:65].bitcast(F32), 1.0)

            # ---- phase A+B: projections, one PSUM bank per accumulation ----
            # One 2MB DMA per (half, t2) pass, alternating HWDGE rings.
            xt_tiles = {}

            def load_tile(half, t2):
                xt_t = xpool.tile([128, ND, 512], dt_mm, tag="xt",
                                  name=f"xt{half}_{t2}")
                src = xt_d[:, half * 2048 + t2 * 512:
                           half * 2048 + (t2 + 1) * 512]
                srcr = src.rearrange("(c p) t -> p c t", p=128)
                eng = nc.sync if (half * 4 + t2) % 2 == 0 else nc.scalar
                eng.dma_start(xt_t[:, 0:ND // 2, :], srcr[:, 0:ND // 2, :])
                eng.dma_start(xt_t[:, ND // 2:ND, :], srcr[:, ND // 2:ND, :])
                xt_tiles[(half, t2)] = xt_t

            def q_pass(t2):
                pq = qpool.tile([64, 512], F32, tag="pa", bufs=2, name=f"pq{t2}")
                for d in range(ND):
                    nc.tensor.matmul(pq[:], wq_sb[:, d, :],
                                     xt_tiles[(0, t2)][:, d, :],
                                     start=(d == 0), stop=(d == ND - 1))
                nc.vector.tensor_scalar_add(qts[t2][:], pq[:], bq_sb[:])

            def kv_pass(half, t2):
                pkv = qpool.tile([128, 512], F32, tag="pa", bufs=2,
                                 name=f"pkv{half}_{t2}")
                for d in range(ND):
                    nc.tensor.matmul(pkv[:], wkv_sb[:, d, :],
                                     xt_tiles[(half, t2)][:, d, :],
                                     start=(d == 0), stop=(d == ND - 1))
                i = half * 4 + t2
                nc.vector.tensor_scalar_add(kvs[i][:], pkv[:], bkv_sb[:])
                if "B" in phases:
                    for cc in range(4):
                        c = i * 4 + cc
                        pvt = qpool.tile([128, 64], dt_mm, tag="pa", bufs=2,
                                         name=f"pvt{c}")
                        nc.tensor.transpose(
                            pvt[:], kvs[i][64:128, cc * 128:(cc + 1) * 128],
                            id_sb[64:128, 64:128])
                        nc.vector.tensor_copy(vau[c][:, 0:64], pvt[:])

            pos = {}

            def attn_group(tqc, g):
                # 4 consecutive s-chunks of the attention pipeline for query
                # chunk tqc, emitted as soon as their K/V chunks exist.
                if "C" not in phases:
                    return
                if tqc not in pos:
                    pos[tqc] = qpool.tile([65, 1024], F32, tag="po", bufs=1,
                                          name=f"po{tqc}")
                po = pos[tqc]
                for s in range(4 * g, 4 * g + 4):
                    ps_t = qpool.tile([128, 1024], F32, tag="ps", bufs=2,
                                      name=f"ps{tqc}_{s}")
                    for j in range(2):
                        nc.tensor.matmul(
                            ps_t[:, j * 512:(j + 1) * 512],
                            kvs[s // 4][0:64, (s % 4) * 128:(s % 4 + 1) * 128],
                            qts[2 * tqc + j][:],
                            start=True, stop=True)
                    et = epool.tile([128, 1024], dt_mm, tag="et",
                                    name=f"et{tqc}_{s}")
                    nc.scalar.activation(et[:], ps_t[:], EXPF, scale=SCALE)
                    for j in range(2):
                        nc.tensor.matmul(
                            po[:, j * 512:(j + 1) * 512],
                            vau[s][:],
                            et[:, j * 512:(j + 1) * 512],
                            start=(s == 0), stop=(s == NS - 1))

            def finish(tqc):
                if "C" not in phases or "D" not in phases:
                    return
                oT = opool.tile([65, 1024], F32, tag="oT", bufs=2, name=f"oT{tqc}")
                nc.vector.tensor_copy(oT[:], pos[tqc][:])
                for j in range(8):
                    jj = tqc * 8 + j
                    pt = qpool.tile([128, 65], F32, tag="pa", bufs=2, name=f"pt{jj}")
                    nc.tensor.transpose(pt[:], oT[:, j * 128:(j + 1) * 128],
                                        id_sb[0:65, 0:65].bitcast(F32))
                    rec = opool.tile([128, 1], F32, tag="rec", bufs=2,
                                     name=f"rec{jj}")
                    nc.vector.reciprocal(rec[:], pt[:, 64:65])
                    nc.vector.tensor_scalar_mul(osb[:, jj, :], pt[:, 0:64], rec[:])
                odst = out_d.rearrange("(j p) o -> p j o", p=128)
                nc.sync.dma_start(odst[:, tqc * 8:(tqc + 1) * 8, :],
                                  osb[:, tqc * 8:(tqc + 1) * 8, :])

            for t2 in range(4):
                load_tile(0, t2)
            for t2 in range(4):
                load_tile(1, t2)
            # interleave attention (tqc=0) with K/V production so ACT starts
            # as soon as the first K/V chunks and qT[0:1024] exist
            q_pass(0)
            kv_pass(0, 0)
            q_pass(1)
            kv_pass(0, 1)
            attn_group(0, 0)
            q_pass(2)
            kv_pass(0, 2)
            attn_group(0, 1)
            q_pass(3)
            kv_pass(0, 3)
            attn_group(0, 2)
            for t2 in range(4):
                kv_pass(1, t2)
                attn_group(0, 3 + t2)
            attn_group(0, 7)
            for g in range(8):
                attn_group(1, g)
            finish(0)
            finish(1)

        if n_iters is None:
            body()
        else:
            with tc.For_i(0, n_iters, 1) as _i:
                body()


def build_program(dt_mm=F32R, phases="ABCD", n_iters=None):
    nc = bacc.Bacc("TRN2", target_bir_lowering=False, debug=False,
                   num_devices=N_CORES)
    io = {
        "xt": nc.dram_tensor("xt", [D_IN, T], dt_mm, kind="ExternalInput").ap(),
        "wkv": nc.dram_tensor("wkv", [D_IN, 128], dt_mm, kind="ExternalInput").ap(),
        "wq": nc.dram_tensor("wq", [D_IN, 64], dt_mm, kind="ExternalInput").ap(),
        "bkv": nc.dram_tensor("bkv", [128, 1], F32, kind="ExternalInput").ap(),
        "bq": nc.dram_tensor("bq", [64, 1], F32, kind="ExternalInput").ap(),
        "ident": nc.dram_tensor("ident", [128, 128], dt_mm, kind="ExternalInput").ap(),
        "out": nc.dram_tensor("out", [TQ, 64], F32, kind="ExternalOutput").ap(),
    }
    with tile.TileContext(nc) as tc:
        emit_body(nc, tc, io, dt_mm, phases=phases, n_iters=n_iters)
    nc.compile()
    return nc


_PROGRAM_CACHE = {}


def get_program(dt_mm=F32R):
    key = str(dt_mm)
    if key not in _PROGRAM_CACHE:
        _PROGRAM_CACHE[key] = build_program(dt_mm)
    return _PROGRAM_CACHE[key]


def make_in_maps(x, Wk, bk, Wq, bq, Wv, bv):
    x = np.asarray(x, dtype=np.float32)
    wkv = np.ascontiguousarray(np.concatenate([Wk, Wv], axis=1), dtype=np.float32)
    wq = np.ascontiguousarray(Wq, dtype=np.float32)
    bkv = np.concatenate([bk, bv]).astype(np.float32).reshape(128, 1)
    bqv = np.asarray(bq, dtype=np.float32).reshape(64, 1)
    ident = np.eye(128, dtype=np.float32)
    in_maps = []
    for c in range(N_CORES):
        b, half = c // 2, c % 2
        xb = x[b]
        own = xb[half * TQ:(half + 1) * TQ].T
        other = xb[(1 - half) * TQ:(2 - half) * TQ].T
        xt = np.ascontiguousarray(np.concatenate([own, other], axis=1))
        in_maps.append({"xt": xt, "wkv": wkv, "wq": wq, "bkv": bkv,
                        "bq": bqv, "ident": ident})
    return in_maps


def assemble(results):
    out = np.empty((B, T, D_OUT), dtype=np.float32)
    for c in range(N_CORES):
        b, half = c // 2, c % 2
        out[b, half * TQ:(half + 1) * TQ, :] = results[c]["out"]
    return out


def kernel(x, Wk, bk, Wq, bq, Wv, bv):
    nc = get_program()
    in_maps = make_in_maps(x, Wk, bk, Wq, bq, Wv, bv)
    res = run_bass_kernel_spmd(nc, in_maps, list(range(N_CORES)))
    return assemble(res.results)
